# revision 1
# baseline (speedup 1.0000x reference)
"""CQT (8-octave, 36 bins/oct) Trainium2 Bass kernel.

Math: the reference's filtfilt (order-10 Butterworth, zero ICs) is exactly
causal convolution with the IIR impulse response h; truncating h at
K=128 taps (decay < 1e-8 rel) makes filtfilt == conv with g = h (corr) h
(255 taps) PLUS an exact right-edge correction (the backward lfilter pass
only sees the truncated forward output y[0..T-1], not its conv tail).
Everything is then FIR convs -> PE matmuls.

Device mapping per signal (2 signals per core, batch-parallel over 8
cores): each stage-o signal s_o lives in SBUF as two interleaved halves
  ILE[p, m] = flat[256*(m-1) + p],  ILO[p, m] = flat[256*(m-1) + 128 + p]
with flat[f] = s_o[f - 135] (135 zeros left, ~600 zeros right).  The
decimation s_{o-1}[m'] = sum_d g[d] s_o[2m'-d] becomes 5 Toeplitz-
stationary matmuls per 512-block chunk whose [128 x N] PSUM output IS the
next stage's interleaved layout (partition-aligned via an output shift of
7 = 135 mod 128), plus tiny fp32 edge matmuls.  CQT octaves 7/6 read
im2col access patterns directly off the halves (contiguous, base-0);
octaves 0-5 DMA a hop-strided replicated tile SL[p,j] = flat[hop*j + p]
from a DRAM bounce of the signal, giving contiguous base-0 matmuls.

Big matmuls run float32r (full PE rate at N>=256); N<256 and edge
matmuls run plain float32 (same 4cyc/row cost at small N, no AP
restrictions).  This walrus build allows only ONE sync wait per
instruction, so a post-pass splits Tile's multi-wait instructions into
single-wait NOP chains.
"""

import numpy as np
from contextlib import ExitStack

import concourse.bass as bass
import concourse.tile as tile
import concourse.mybir as mybir
from concourse.bass_utils import run_bass_kernel_spmd

dt = mybir.dt

# ---------------- problem constants ----------------
B_TOT, T_TOP, N_CORES = 16, 524288, 8
B_PER_CORE = B_TOT // N_CORES
BPO, NF, KW, PAD, FRAMES = 36, 72, 271, 135, 2048
KIR = 128
HOPS = [2, 4, 8, 16, 32, 64, 128, 256]
TS = [T_TOP >> (7 - o) for o in range(8)]
SL_OCTS = (0, 1, 2, 3, 4, 5)
SL_COLS = 2256          # shared SL tile width (max needed 2247)
# debug/bisect switches (module-level so build variants are easy)
DBG_CQT_OCTS = set(range(8))
DBG_DECIM = True
DBG_OUT = True
DBG_SL = True


# ---------------- host-side constants ----------------
def _impulse_response(b, a):
    b = (b / a[0]).astype(np.float64)
    a = (a / a[0]).astype(np.float64)
    h = np.zeros(KIR)
    x = np.zeros(KIR)
    x[0] = 1.0
    for n in range(KIR):
        acc = 0.0
        for i in range(min(11, n + 1)):
            acc += b[i] * x[n - i]
        for i in range(1, min(11, n + 1)):
            acc -= a[i] * h[n - i]
        h[n] = acc
    return h


def make_consts(cqt_kernels, iir_num, iir_den):
    ker = np.asarray(cqt_kernels, np.float64)[:, 0, :]      # (72, 271)
    h = _impulse_response(np.asarray(iir_num, np.float64),
                          np.asarray(iir_den, np.float64))
    g = np.correlate(h, h, "full")                          # (255,)

    def gd(d):
        d = int(d)
        return g[d + KIR - 1] if -KIR < d < KIR else 0.0

    E = np.zeros((KIR - 1, KIR))
    for i in range(KIR - 1):
        for p in range(KIR):
            mmax = min(KIR - 1 - i, KIR - 1 - p)
            if mmax >= 1:
                m = np.arange(1, mmax + 1)
                E[i, p] = (h[i + m] * h[m + p]).sum()

    KB = np.zeros((3, 128, NF))
    for c in range(3):
        for r in range(128):
            tap = 128 * c + r
            if tap < KW:
                KB[c, r, :] = ker[:, tap]

    # decim Toeplitz: psum[i, n] = s_next[128n + i - 7],
    # window chunk c: rhs row k <-> flat 256n - 128 + 128c + k
    #   => s_o index 256n - 263 + 128c + k ; d = 2i + 249 - 128c - k
    G = np.zeros((5, 128, 128))
    for c in range(5):
        for k in range(128):
            for i in range(128):
                G[c, k, i] = gd(2 * i + 249 - 128 * c - k)

    # EA: edge corr for outputs q in [0,57): psum[71+q, C-1] -= E[2q+1,p] x[T-1-p]
    #   rhs0 = ILE col T/256+1: row r <-> x[T-135+r]  => p = 134-r (r in [7,128))
    #   rhs1 = ILO col T/256+1: row r <-> x[T-7+r]    => p = 6-r   (r in [0,7))
    # out m = T/2-1-q has corr index i = 2q+1; psum row = m-T/2+135 = 134-q.
    # q in [0,7) are the tail outputs (EB); q=63 has zero corr (i=127).
    EA = np.zeros((2, 128, 128))
    for q in range(7, 63):
        i = 2 * q + 1
        m = 134 - q
        for p in range(7, 128):
            EA[0, 134 - p, m] = -E[i, p]
        for p in range(0, 7):
            EA[1, 6 - p, m] = -E[i, p]

    # EB: full last-7 outputs s_next[T/2-7+ii] -> psum_tail[ii, 0]
    #   coeff[j] multiplies x[T-1-j]: conv part g[d] at j = d + 13 - 2*ii,
    #   minus edge corr E[2(57+ii)+1, j] for j < 128.
    #   rhs0 = ILO col T/256:   row r <-> x[T-263+r] => j = 262 - r
    #   rhs1 = ILE col T/256+1: row r <-> x[T-135+r] => j = 134 - r
    #   rhs2 = ILO col T/256+1: row r <-> x[T-7+r]   => j = 6 - r (r<7)
    EB = np.zeros((3, 128, 128))
    for ii in range(7):
        i_edge = 13 - 2 * ii
        coeff = np.zeros(300)
        for d in range(-127, 128):
            j = d + 13 - 2 * ii
            if 0 <= j < 300:
                coeff[j] += gd(d)
        if i_edge < KIR - 1:
            for p in range(128):
                coeff[p] -= E[i_edge, p]
        for j in range(0, 7):
            EB[2, 6 - j, ii] += coeff[j]
        for j in range(7, 135):
            EB[1, 134 - j, ii] += coeff[j]
        for j in range(135, 263):
            EB[0, 262 - j, ii] += coeff[j]

    blobs = ([KB[c] for c in range(3)] + [G[c] for c in range(5)]
             + [EA[0], EA[1], EB[0], EB[1], EB[2], np.eye(128),
                np.zeros((128, 8))])
    offs = np.cumsum([0] + [bl.shape[1] for bl in blobs])
    blob = np.concatenate(blobs, axis=1).astype(np.float32)
    return blob, offs


# ---------------- wait splitting post-pass ----------------
def _split_multi_waits(nc):
    n_split = 0
    for f in nc.m.functions:
        for bb in f.blocks:
            insts = bb.instructions
            out = []
            changed = False
            for inst in insts:
                si = inst.sync_info
                ow = list(si.on_wait) if (si is not None and si.on_wait) else []
                if len(ow) > 1:
                    for w in ow[:-1]:
                        nop = mybir.InstNoOp(name=f"wsplit_{n_split}",
                                             ins=[], outs=[])
                        n_split += 1
                        nop.engine = inst.engine
                        nop.sync_info = mybir.SyncInfo(on_wait=[w],
                                                       on_update=[])
                        out.append(nop)
                    si.on_wait = [ow[-1]]
                    changed = True
                out.append(inst)
            if changed:
                insts.clear()
                insts.extend(out)
    return n_split


# ---------------- device kernel ----------------
def build_nc(nconst_cols, repeat=1, timing=False):
    f32r, f32 = dt.float32r, dt.float32
    nc = bass.Bass()
    X = nc.declare_dram_parameter("x", [B_PER_CORE, T_TOP], f32,
                                  isOutput=False)
    CONST = nc.declare_dram_parameter("consts", [128, nconst_cols], f32,
                                      isOutput=False)
    if timing:
        OUT = nc.dram_tensor("out_scratch", [B_PER_CORE, 2, 288, FRAMES],
                             f32)
        DONE = nc.declare_dram_parameter("done", [128, 4], f32,
                                         isOutput=True)
    else:
        OUT = nc.declare_dram_parameter("out", [B_PER_CORE, 2, 288, FRAMES],
                                        f32, isOutput=True)
        DONE = None
    scratch = {}
    for b in range(B_PER_CORE):
        for o in SL_OCTS:
            scratch[(b, o)] = nc.dram_tensor(f"scr_{b}_{o}",
                                             [TS[o] + 768], f32r)

    with tile.TileContext(nc) as tc, ExitStack() as ctx:
        cpool = ctx.enter_context(tc.tile_pool(name="consts", bufs=1))
        ilpool = ctx.enter_context(tc.tile_pool(name="il", bufs=1))
        segpool = ctx.enter_context(tc.tile_pool(name="seg", bufs=1))  # per-b tag -> 1 buf each
        xfpool = ctx.enter_context(tc.tile_pool(name="xf", bufs=6))
        xxpool = ctx.enter_context(tc.tile_pool(name="xx", bufs=3))
        pq = ctx.enter_context(tc.tile_pool(name="pq", bufs=2, space="PSUM"))
        pd = ctx.enter_context(tc.tile_pool(name="pd", bufs=2, space="PSUM"))
        pt = ctx.enter_context(tc.tile_pool(name="pt", bufs=1, space="PSUM"))
        pw = ctx.enter_context(tc.tile_pool(name="pw", bufs=3, space="PSUM"))
        stpool = ctx.enter_context(tc.tile_pool(name="stage", bufs=6))

        consts = cpool.tile([128, nconst_cols], f32r)
        nc.sync.dma_start(consts[:], CONST[:].bitcast(f32r))

        # const column offsets: 3x72 (KB), 5x128 (G), 2x128 (EA), 3x128 (EB)
        CO = np.cumsum([0] + [NF] * 3 + [128] * 11 + [8]).tolist()

        def KBc(c, K):
            return consts[0:K, CO[c]:CO[c] + NF]

        def Gc(c):
            return consts[:, CO[3 + c]:CO[3 + c] + 128]

        def EAc(c):
            return consts[:, CO[8 + c]:CO[8 + c] + 128].bitcast(f32)

        def EBc(c):
            return consts[:, CO[10 + c]:CO[10 + c] + 128].bitcast(f32)

        IDOFF = CO[13]
        ZOFF = CO[14]

        def ident():
            return consts[:, IDOFF:IDOFF + 128]

        def zeros(ap):
            n = ap.shape[-1]
            if ap.partition_size() == 128:
                nc.vector.tensor_copy(ap, consts[:, ZOFF:ZOFF + n])
            else:
                nc.vector.tensor_copy(
                    ap, consts[0:ap.partition_size(), ZOFF:ZOFF + n])

        for rep in range(repeat):
            all_steps = [_emit_signal(nc, X, OUT, scratch, b,
                                      ilpool, segpool, xfpool, xxpool,
                                      stpool, pq, pd, pt, pw,
                                      KBc, Gc, EAc, EBc, zeros, ident, rep)
                         for b in range(B_PER_CORE)]
            for pair in zip(*all_steps):
                for s in pair:
                    s()
        if timing:
            nc.sync.dma_start(DONE[:, :], consts[:, 0:4].bitcast(dt.float32))
    _split_multi_waits(nc)
    return nc


def _emit_signal(nc, X, OUT, scratch, b, ilpool, segpool, xfpool, xxpool,
                 stpool, pq, pd, pt, pw,
                 KBc, Gc, EAc, EBc, zeros, ident, rep):
    f32r, f32 = dt.float32r, dt.float32

    ILE, ILO, NCOL = {}, {}, {}
    for o in range(8):
        cols = TS[o] // 256 + 4
        NCOL[o] = cols
        ILE[o] = ilpool.tile([128, cols], f32r, tag=f"ile{o}_{b}", name=f"ile{o}_{b}")
        ILO[o] = ilpool.tile([128, cols], f32r, tag=f"ilo{o}_{b}", name=f"ilo{o}_{b}")

    def memset_pads(o):
        e, od, cols = ILE[o], ILO[o], NCOL[o]
        zeros(e[:, 0:2])             # flat [-256, 0) U [0,128)
        zeros(od[:, 0:1])            # flat [-128, 0)
        zeros(od[0:7, 1:2])          # flat [128, 135)
        zeros(e[:, cols - 2:cols])
        zeros(od[:, cols - 3:cols])

    # ---------- load x into IL7 via SEG tile + PE transposes ----------
    # seg[p, q] = x[4096 p + q - 7]; tr_v[j, p] = seg[p, 128v + j] lands at
    # IL flat 128(32p + v + 1) + j: partition j, col_flat 32p+v+1.
    def load_x():
      memset_pads(7)
      C7 = TS[7] // 256
      seg = segpool.tile([128, 4096], f32r, tag=f"seg_{b}", name="seg_t")
      zeros(seg[0:1, 0:7])
      nc.sync.dma_start(seg[:, 7:4096],
                        bass.AP(X, b * T_TOP,
                                [[4096, 128], [1, 4089]]).bitcast(f32r))
      nc.sync.dma_start(seg[1:128, 0:7],
                        bass.AP(X, b * T_TOP + 4089,
                                [[4096, 127], [1, 7]]).bitcast(f32r))
      for v in range(32):
          ptr = pw.tile([128, 128], f32r, tag="tr", name="tr_ps")
          nc.tensor.transpose(ptr[:, :], seg[:, 128 * v:128 * v + 128],
                              ident())
          if v % 2 == 1:
              dst, m0 = ILE[7], (v + 1) // 2 + 1
          else:
              dst, m0 = ILO[7], v // 2 + 1
          nc.vector.tensor_copy(dst[:, m0:m0 + 16 * 127 + 1:16], ptr[:, :])
      # last 7 samples x[T-7..T) -> ILO7[0:7, 1+C7]
      nc.sync.dma_start(ILO[7][0:7, 1 + C7:2 + C7],
                        bass.AP(X, b * T_TOP + T_TOP - 7,
                                [[1, 7], [1, 1]]).bitcast(f32r))

    def emit_cqt_xf(o):
        """CQT via frame-major im2col rows + PE transpose (octaves 0-5).

        Xf row i = 272 contiguous samples at scratch flat2 hop*(t0+i)+128
        (= sample hop*(t0+i)-135).  Transposed 128-col blocks build the
        tap-major X in SBUF, then 3 matmuls per 512-frame chunk."""
        hop = HOPS[o]
        scr = scratch[(b, o)]
        for nchunk in range(4):
            xx = xxpool.tile([128, 1536], f32r, tag="xx", name="xx_t")
            for fc in range(4):
                t0 = nchunk * 512 + fc * 128
                xf = xfpool.tile([128, 272], f32r, tag="xf", name="xf_t")
                nc.sync.dma_start(
                    xf[:, :],
                    bass.AP(scr, hop * t0 + 128, [[hop, 128], [1, 272]]))
                for c, K in ((0, 128), (1, 128), (2, 15)):
                    ptr = pw.tile([128, 128], f32r, tag="tr", name="tr_ps")
                    nc.tensor.transpose(ptr[0:K, :],
                                        xf[:, 128 * c:128 * c + K], ident())
                    dstx = xx[0:K, 512 * c + 128 * fc:
                              512 * c + 128 * fc + 128]
                    if c == 0:
                        nc.scalar.copy(dstx, ptr[0:K, 0:128])
                    else:
                        nc.vector.tensor_copy(dstx, ptr[0:K, 0:128])
            t0 = nchunk * 512
            ps = pq.tile([NF, 512], f32, tag="cqt", name="cqt_ps")
            for c, K in ((0, 128), (1, 128), (2, 15)):
                nc.tensor.matmul(ps[:, :], KBc(c, K),
                                 xx[0:K, 512 * c:512 * c + 512],
                                 start=(c == 0), stop=(c == 2))
            _emit_out(nc, stpool, OUT, b, o, ps, t0, 1)

    def emit_cqt_il(o):
        """CQT off IL halves (o in {6,7})."""
        if o == 7:
            for nchunk in range(4):
                u0 = nchunk * 512
                ps = pq.tile([NF, 512], f32, tag="cqt", name="cqt_ps")
                for idx, (c, half, dm) in enumerate(
                        ((0, ILE[7], 1), (1, ILO[7], 1), (2, ILE[7], 2))):
                    K = (128, 128, 15)[c]
                    nc.tensor.matmul(ps[:, :], KBc(c, K),
                                     half[0:K, u0 + dm:u0 + dm + 512],
                                     start=(idx == 0), stop=(idx == 2))
                _emit_out(nc, stpool, OUT, b, o, ps, u0, 1)
            return
        # o == 6: two residues (t = 2u + r); stage both into one tile so
        # the out-DMA stays contiguous (strided DRAM writes cost 4B/desc).
        plans = [((0, ILE[6], 1), (1, ILO[6], 1), (2, ILE[6], 2)),
                 ((0, ILO[6], 1), (1, ILE[6], 2), (2, ILO[6], 2))]
        for nchunk in range(2):
            u0 = nchunk * 512
            st = stpool.tile([NF, 1024], f32, tag="stage6", name="stage6_t")
            for r in range(2):
                ps = pq.tile([NF, 512], f32, tag="cqt", name="cqt_ps")
                for idx, (c, half, dm) in enumerate(plans[r]):
                    K = (128, 128, 15)[c]
                    nc.tensor.matmul(ps[:, :], KBc(c, K),
                                     half[0:K, u0 + dm:u0 + dm + 512],
                                     start=(idx == 0), stop=(idx == 2))
                nc.vector.tensor_copy(st[:, r:r + 1023:2], ps[0:NF, :])
            for c in range(2):
                off = ((b * 2 + c) * 288 + 36 * 6) * FRAMES + 2 * u0
                dst = bass.AP(OUT, off, [[FRAMES, 36], [1, 1024]])
                nc.sync.dma_start(dst, st[c * 36:(c + 1) * 36, :])

    def emit_decim(o):
        """s_o -> s_{o-1}: Toeplitz matmuls + edge, PSUM -> IL halves."""
        on = o - 1
        Tn = TS[o]
        memset_pads(on)
        cT = Tn // 256 + 1   # IL col holding flat [T, T+128) / [T+128, T+256)
        nblocks = Tn // 256  # main blocks n in [0, nblocks)
        n0 = 0
        while n0 < nblocks:
            N = min(512, nblocks - n0)
            use_r = N >= 256
            last = (n0 + N == nblocks)
            ps = pd.tile([128, 512], f32, tag="dec", name="dec_ps")
            for c in range(5):
                half = ILO[o] if c % 2 == 0 else ILE[o]
                col = n0 + (c + 1) // 2
                rhs = half[:, col:col + N]
                lhs = Gc(c)
                if not use_r:
                    rhs, lhs = rhs.bitcast(f32), lhs.bitcast(f32)
                nc.tensor.matmul(ps[:, 0:N], lhs, rhs,
                                 start=(c == 0),
                                 stop=(c == 4 and not last))
            if last:
                # edge corr accumulates into psum col N-1 (= block nblocks-1)
                nc.tensor.matmul(ps[:, N - 1:N], EAc(0),
                                 ILE[o][:, cT:cT + 1].bitcast(f32),
                                 start=False, stop=False)
                nc.tensor.matmul(ps[:, N - 1:N], EAc(1),
                                 ILO[o][:, cT:cT + 1].bitcast(f32),
                                 start=False, stop=True)
            # psum block n -> flat' 128(n+1)+i:
            #   n even -> ILO'[i, n//2 + 1]; n odd -> ILE'[i, (n+1)//2 + 1]
            for par in range(2):  # par = n parity within this chunk
                js = [j for j in range(N) if (n0 + j) % 2 == par]
                if not js:
                    continue
                j0, cnt = js[0], len(js)
                n_first = n0 + j0
                if par == 0:
                    dst = ILO[on][:, n_first // 2 + 1:n_first // 2 + 1 + cnt]
                else:
                    dst = ILE[on][:, (n_first + 1) // 2 + 1:
                                  (n_first + 1) // 2 + 1 + cnt]
                if n_first == 0:
                    # block 0 rows [0,7) are left-pad: copy the full column,
                    # then re-zero the pad rows (flat' [128,135))
                    nc.vector.tensor_copy(dst[:, 0:1], ps[:, 0:1])
                    zeros(dst[0:7, 0:1])
                    if cnt > 1:
                        nc.vector.tensor_copy(
                            dst[:, 1:cnt], ps[:, j0 + 2:j0 + 2 * cnt - 1:2])
                elif cnt > 1:
                    nc.vector.tensor_copy(dst, ps[:, j0:j0 + 2 * cnt - 1:2])
                else:
                    nc.vector.tensor_copy(dst, ps[:, j0:j0 + 1])
            n0 += N
        # tail: last 7 outputs -> ILO'[0:7, Tn//512 + 1]
        pst = pt.tile([128, 1], f32, tag="tail", name="tail_ps")
        nc.tensor.matmul(pst[:, :], EBc(0),
                         ILO[o][:, cT - 1:cT].bitcast(f32),
                         start=True, stop=False)
        nc.tensor.matmul(pst[:, :], EBc(1),
                         ILE[o][:, cT:cT + 1].bitcast(f32),
                         start=False, stop=False)
        nc.tensor.matmul(pst[:, :], EBc(2),
                         ILO[o][:, cT:cT + 1].bitcast(f32),
                         start=False, stop=True)
        tcol = Tn // 512 + 1
        nc.vector.tensor_copy(ILO[on][0:7, tcol:tcol + 1], pst[0:7, 0:1])

    def store_scratch(o):
        """Store s_o (IL halves) to flat scratch via PE transposes.

        Transposed row j of an IL col-range holds 128 contiguous samples
        (flat 256(col-1) [+128 for ILO]); scratch flat2 = flat + 128, so
        per-partition contiguous 512B store descriptors.
        ILE cols [1, ncol) -> offset 256(done-1)+128; ILO cols [0, ncol)
        -> offset 256*done."""
        scr = scratch[(b, o)]
        ncol = NCOL[o]
        for half_i, half, c0, cend in ((0, ILE[o], 1, ncol),
                                       (1, ILO[o], 0, ncol - 1)):
            done = c0
            while done < cend:
                cc = min(128, cend - done)
                ptr = pw.tile([128, 128], f32r, tag="tr", name="tr_ps")
                nc.tensor.transpose(ptr[0:cc, :], half[:, done:done + cc],
                                    ident())
                sg = xfpool.tile([128, 128], f32r, tag="sg", name="sg_t")
                nc.vector.tensor_copy(sg[0:cc, :], ptr[0:cc, :])
                off = (256 * (done - 1) + 128 if half_i == 0
                       else 256 * done)
                nc.sync.dma_start(
                    bass.AP(scr, off, [[256, cc], [1, 128]]),
                    sg[0:cc, :])
                done += cc

    # ---------- main cascade as interleavable steps ----------
    steps = [load_x]
    if 7 in DBG_CQT_OCTS:
        steps.append(lambda: emit_cqt_il(7))

    def stage(o):
        def run():
            if DBG_DECIM:
                emit_decim(o)
            if o - 1 in SL_OCTS:
                if DBG_DECIM and DBG_SL and (o - 1) in DBG_CQT_OCTS:
                    store_scratch(o - 1)
                    emit_cqt_xf(o - 1)
            elif o - 1 == 6 and 6 in DBG_CQT_OCTS:
                emit_cqt_il(6)
        return run

    for o in range(7, 0, -1):
        steps.append(stage(o))
    return steps


def _emit_out(nc, stpool, OUT, b, o, ps, t0, stride):
    """Copy a [72, N] CQT psum tile to SBUF, DMA to
    OUT[b, :, 36o:36o+36, t0::stride]."""
    N = ps.shape[1]
    st = stpool.tile([NF, N], dt.float32, tag="stage", name="stage_t")
    nc.vector.tensor_copy(st[:, :], ps[0:NF, :])
    if not DBG_OUT:
        return
    for c in range(2):
        off = ((b * 2 + c) * 288 + 36 * o) * FRAMES + t0
        dst = bass.AP(OUT, off, [[FRAMES, 36], [stride, N]])
        nc.sync.dma_start(dst, st[c * 36:(c + 1) * 36, :])


# ---------------- public entry ----------------
def kernel(x, cqt_kernels, iir_num, iir_den):
    x = np.ascontiguousarray(np.asarray(x, np.float32))
    blob, offs = make_consts(cqt_kernels, iir_num, iir_den)
    nc = build_nc(blob.shape[1])
    in_maps = [{"x": x[2 * i:2 * i + 2], "consts": blob}
               for i in range(N_CORES)]
    res = run_bass_kernel_spmd(nc, in_maps, list(range(N_CORES)))
    out = np.concatenate([res.results[i]["out"] for i in range(N_CORES)],
                         axis=0)
    return np.asarray(out, np.float32)


# ---------------- timing harness (repeat-slope) ----------------
def _make_runner(nc, in_maps):
    import jax
    from jax.sharding import Mesh, PartitionSpec
    from jax.experimental.shard_map import shard_map
    from concourse.bass2jax import (install_neuronx_cc_hook,
                                    partition_id_tensor, _bass_exec_p)
    install_neuronx_cc_hook()
    partition_name = (nc.partition_id_tensor.name
                      if nc.partition_id_tensor else None)
    in_names, out_names, out_avals, zero_outs = [], [], [], []
    for alloc in nc.m.functions[0].allocations:
        if not isinstance(alloc, mybir.MemoryLocationSet):
            continue
        name = alloc.memorylocations[0].name
        if alloc.kind == "ExternalInput":
            if name != partition_name:
                in_names.append(name)
        elif alloc.kind == "ExternalOutput":
            out_names.append(name)
            shape = tuple(alloc.tensor_shape)
            dtype = mybir.dt.np(alloc.dtype)
            out_avals.append(jax.core.ShapedArray(shape, dtype))
            zero_outs.append(np.zeros(shape, dtype))
    n_params = len(in_names)
    n_outs = len(out_avals)
    all_names = in_names + out_names
    if partition_name is not None:
        all_names.append(partition_name)
    donate = tuple(range(n_params, n_params + n_outs))

    def _body(*args):
        operands = list(args)
        if partition_name is not None:
            operands.append(partition_id_tensor())
        outs = _bass_exec_p.bind(
            *operands, out_avals=tuple(out_avals), in_names=tuple(all_names),
            out_names=tuple(out_names), lowering_input_output_aliases=(),
            sim_require_finite=True, sim_require_nnan=True, nc=nc)
        return tuple(outs)

    devices = jax.devices()[:N_CORES]
    mesh = Mesh(np.asarray(devices), ("core",))
    in_specs = (PartitionSpec("core"),) * (n_params + n_outs)
    out_specs = (PartitionSpec("core"),) * n_outs
    sharded = jax.jit(shard_map(_body, mesh=mesh, in_specs=in_specs,
                                out_specs=out_specs, check_rep=False),
                      donate_argnums=donate, keep_unused=True)
    per_core = [[np.asarray(m[nm]) for nm in in_names] for m in in_maps]
    concat_in = [np.concatenate([per_core[c][i] for c in range(N_CORES)],
                                axis=0) for i in range(n_params)]
    zero_shapes = [(N_CORES * z.shape[0], *z.shape[1:]) for z in zero_outs]
    return sharded, concat_in, zero_shapes


def time_kernel(inputs, reps=(1, 4), calls=10):
    import time as _time
    import jax
    x = np.ascontiguousarray(np.asarray(inputs["x"], np.float32))
    blob, _ = make_consts(inputs["cqt_kernels"], inputs["iir_num"],
                          inputs["iir_den"])
    in_maps = [{"x": x[2 * i:2 * i + 2], "consts": blob}
               for i in range(N_CORES)]
    from jax.sharding import Mesh, PartitionSpec, NamedSharding
    # paired alternating measurement: tunnel-latency drift cancels in the
    # per-round (wall_R1 - wall_R0)/(R1-R0) differences.
    runners = {}
    for R in reps:
        nc = build_nc(blob.shape[1], repeat=R, timing=True)
        sharded, concat_in, zshapes = _make_runner(nc, in_maps)
        mesh = Mesh(np.asarray(jax.devices()[:N_CORES]), ("core",))
        sh = NamedSharding(mesh, PartitionSpec("core"))
        din = [jax.device_put(a, sh) for a in concat_in]
        jax.block_until_ready(din)
        jax.block_until_ready(
            sharded(*din, *[np.zeros(s, np.float32) for s in zshapes]))
        runners[R] = (sharded, din, zshapes)
    r0, r1 = reps
    diffs = []
    for _ in range(calls):
        ws = {}
        for R in reps:
            sharded, din, zshapes = runners[R]
            zs = [np.zeros(s, np.float32) for s in zshapes]
            t0 = _time.perf_counter()
            jax.block_until_ready(sharded(*din, *zs))
            ws[R] = _time.perf_counter() - t0
        diffs.append((ws[r1] - ws[r0]) / (r1 - r0))
    print("  per-body diffs (us):",
          np.round(np.array(diffs) * 1e6).astype(int))
    return float(np.median(diffs)) * 1e9

